# revision 1
# baseline (speedup 1.0000x reference)
"""Trainium2 Bass kernel for nn_MultiHeadAttention_42752104464925.

Multi-head attention (VITS-style) with windowed relative position embeddings
(window=4, heads_share=True).

Math notes
----------
With L=1024, WIN=4, the relative-key logits term rel_to_abs(q_scaled @ rel_k^T)
is a 9-diagonal band:   scores[t,s] += q_scaled[t] . emb_k[s-t+4]   (|s-t|<=4)
and the relative-value term is:
  out[t] += sum_j p[t, t+j-4] * emb_v[j]   (0 <= t+j-4 < L)

Sharding: 8 cores = 4 batches x 2 head-groups (6 heads each). Each core
computes QKV for its 384 channels, attention for its 6 heads, and a partial
output projection Wo[:, slice] @ merged. Host sums the two partials per batch.

Device layout per head: scores are computed TRANSPOSED (S^T[s,t], s on
partitions) so that A@V needs no transpose of the attention weights; the
softmax denominator L[t] (a partition-dim sum) comes for free from a ones
column appended to the V^T stationary operand (M=65).

The diagonal bands are applied via small DRAM staging buffers read back with
overlapping linear access patterns ("staircase" reads). The zero padding those
reads rely on is supplied as host-side zero input tensors (transferred before
kernel execution, so they cost no device time).
"""

import math

import numpy as np

import concourse.bacc as bacc
import concourse.bass as bass
import concourse.mybir as mybir
import concourse.tile as tile
from concourse import library_config
from concourse.bass_utils import run_bass_kernel_spmd

# Problem constants (hardcoded per harness contract).
B, C, T, H, KC, WIN = 4, 768, 1024, 12, 64, 4
HL = 6            # heads per core
CL = HL * KC      # 384 local channels
NSUB = C // 128   # 6 k-subtiles over C
LSUB = CL // 128  # 3 subtiles over CL
NCH = T // 128    # 8 s-chunks
NB = 9            # band width (2*WIN+1)
RT_ROW = 264      # rtpad row stride: cols [127,136) hold Rt[t, j], rest zero
W2_ROWS, W2_COLS = 144, 136  # padded expS window staging (8 zero rows each end)

F32 = mybir.dt.float32
AF = mybir.ActivationFunctionType
ALU = mybir.AluOpType

# Matmul input dtype. float32r streams at full PE rate (1 cyc/row for N>=256)
# with fp32 storage; plain float32 runs at 1/4 rate.
MM_DT = mybir.dt.float32r

# debug toggles (bisect)
EN_BANDK = True    # W-window read + add into scores
EN_W2 = True       # expS window staging writes
EN_GATHER = True   # pd gather accum DMAs
EN_RT = True       # rt staging writes
EN_NORM = True     # reciprocal+broadcast normalize


def _mm(x):
    return x if MM_DT == F32 else x.bitcast(MM_DT)


def _f32(x):
    return x if MM_DT == F32 else x.bitcast(F32)


def _raw(t_ap, off, dims):
    """Raw element-offset AP into (the tensor behind) an AP."""
    return bass.AP(tensor=t_ap.tensor, offset=t_ap.offset + off, ap=dims)


def _chunk_window(c):
    """Clipped t-window [t_lo, t_lo+w) for s-chunk c; q0 = offset into the
    unclipped 136-wide window starting at t0 = 128c - 4."""
    t0 = 128 * c - WIN
    t_lo = max(0, t0)
    q0 = t_lo - t0
    w = min(T, t0 + 136) - t_lo
    return t_lo, q0, w


def build_program():
    nc = bacc.Bacc("TRN2", target_bir_lowering=False, debug=False,
                   enable_asserts=True)

    # ---- I/O ----
    xb = nc.dram_tensor("xb", [C, T], F32, kind="ExternalInput")
    cb = nc.dram_tensor("cb", [C, T], F32, kind="ExternalInput")
    wqt = nc.dram_tensor("wqt", [C, CL], F32, kind="ExternalInput")
    wkt = nc.dram_tensor("wkt", [C, CL], F32, kind="ExternalInput")
    wvt = nc.dram_tensor("wvt", [C, CL], F32, kind="ExternalInput")
    wot = nc.dram_tensor("wot", [CL, C], F32, kind="ExternalInput")
    bq2 = nc.dram_tensor("bq2", [128, LSUB], F32, kind="ExternalInput")
    bk2 = nc.dram_tensor("bk2", [128, LSUB], F32, kind="ExternalInput")
    bvr = nc.dram_tensor("bvr", [128, CL], F32, kind="ExternalInput")
    ekt2 = nc.dram_tensor("ekt2", [128, NB], F32, kind="ExternalInput")
    ev9 = nc.dram_tensor("ev9", [NB, KC], F32, kind="ExternalInput")
    ones8 = nc.dram_tensor("ones8", [128, NCH, 2], F32, kind="ExternalInput")
    # zero-padded staging buffers (host supplies zeros; device writes data)
    rtp = [nc.dram_tensor(f"rtp{h}", [T * RT_ROW], F32, kind="ExternalInput")
           for h in range(HL)]
    w2 = [nc.dram_tensor(f"w2_{i}", [W2_ROWS * W2_COLS], F32,
                         kind="ExternalInput") for i in range(HL * NCH)]
    outp = nc.dram_tensor("outp", [C, T], F32, kind="ExternalOutput")

    with tile.TileContext(nc) as tc:
        nc.gpsimd.load_library(library_config.attn)
        with tc.tile_pool(name="persist", bufs=1) as pp:
            # persistent SBUF
            q_sb = pp.tile([128, LSUB, T], MM_DT, tag="q_sb")
            k_sb = pp.tile([128, LSUB, T], MM_DT, tag="k_sb")
            vt = [pp.tile([128, NCH, KC + 2], MM_DT, tag=f"vt{h}", name=f"vt{h}")
                  for h in range(HL)]
            wo_sb = pp.tile([128, LSUB, C], MM_DT, tag="wo_sb")
            merged = pp.tile([128, LSUB, T], MM_DT, tag="merged")
            ekt_sb = pp.tile([128, NB], F32, tag="ekt_sb")
            ev_sb = pp.tile([NB, KC], F32, tag="ev_sb")
            bq_sb = pp.tile([128, LSUB], F32, tag="bq_sb")
            bk_sb = pp.tile([128, LSUB], F32, tag="bk_sb")
            bv_sb = pp.tile([128, CL], F32, tag="bv_sb")

            nc.sync.dma_start(wo_sb[:], _mm(wot.ap().rearrange(
                "(s p) m -> p s m", p=128)))
            nc.sync.dma_start(ekt_sb[:], ekt2.ap())
            nc.sync.dma_start(ev_sb[:], ev9.ap())
            nc.sync.dma_start(bq_sb[:], bq2.ap())
            nc.sync.dma_start(bk_sb[:], bk2.ap())
            nc.sync.dma_start(bv_sb[:], bvr.ap())

            # ---------------- Phase A: projections ----------------
            with tc.tile_pool(name="pa", bufs=1) as pa, \
                 tc.tile_pool(name="pa_ps", bufs=3, space="PSUM") as pa_ps, \
                 tc.tile_pool(name="pa_ps2", bufs=2, space="PSUM") as pa_ps2, \
                 tc.tile_pool(name="pa_ps3", bufs=2, space="PSUM") as pa_ps3:
                x_sb = pa.tile([128, NSUB, T], MM_DT, tag="x_sb")
                c_sb = pa.tile([128, NSUB, T], MM_DT, tag="c_sb")
                wq_sb = pa.tile([128, NSUB, CL], MM_DT, tag="wq_sb")
                wk_sb = pa.tile([128, NSUB, CL], MM_DT, tag="wk_sb")
                wv_sb = pa.tile([128, NSUB, CL], MM_DT, tag="wv_sb")
                nc.sync.dma_start(x_sb[:], _mm(xb.ap().rearrange(
                    "(s p) t -> p s t", p=128)))
                nc.sync.dma_start(c_sb[:], _mm(cb.ap().rearrange(
                    "(s p) t -> p s t", p=128)))
                nc.sync.dma_start(wq_sb[:], _mm(wqt.ap().rearrange(
                    "(s p) m -> p s m", p=128)))
                nc.sync.dma_start(wk_sb[:], _mm(wkt.ap().rearrange(
                    "(s p) m -> p s m", p=128)))
                nc.sync.dma_start(wv_sb[:], _mm(wvt.ap().rearrange(
                    "(s p) m -> p s m", p=128)))

                # Q and K: out[dl, t] = sum_c W*T[c, dl] * x[c, t]  (+bias)
                for dst, wsb, src, bias in ((q_sb, wq_sb, x_sb, bq_sb),
                                            (k_sb, wk_sb, c_sb, bk_sb)):
                    for m in range(LSUB):
                        for n in range(2):
                            ps = pa_ps.tile([128, 512], F32, tag="qk_ps")
                            for k in range(NSUB):
                                nc.tensor.matmul(
                                    ps[:],
                                    wsb[:, k, 128 * m:128 * (m + 1)],
                                    src[:, k, 512 * n:512 * (n + 1)],
                                    start=(k == 0), stop=(k == NSUB - 1))
                            # fused copy+bias on ACT (idle in phase A)
                            nc.scalar.activation(
                                dst[:, m, 512 * n:512 * (n + 1)], ps[:],
                                AF.Identity, bias=bias[:, m:m + 1])

                # Rt[t, j] = q_scaled[t] . emb_k[j]; stage into rtpad cols
                # [127,136). (q_sb already has 1/sqrt(KC) folded via wqt.)
                rts = [pa.tile([128, NCH, NB], F32, tag=f"rts{h}", name=f"rts{h}")
                       for h in range(HL)]
                for h in range(HL):
                    rb = 64 * (h % 2)
                    sub = h // 2
                    for c in range(NCH):
                        rt_ps = pa_ps3.tile([128, NB], F32, tag="rt_ps")
                        nc.tensor.matmul(
                            rt_ps[:],
                            _f32(q_sb[rb:rb + 64, sub, 128 * c:128 * (c + 1)]),
                            ekt_sb[rb:rb + 64, :],
                            start=True, stop=True)
                        nc.vector.tensor_copy(rts[h][:, c, :], rt_ps[:])
                    if EN_RT:
                        nc.sync.dma_start(
                            _raw(rtp[h].ap(), 127,
                                 [[RT_ROW, 128], [RT_ROW * 128, NCH], [1, NB]]),
                            rts[h][:])

                # V^T: out[s, dl] = sum_c c_b[c, s] * WvT[c, dl] (+bias),
                # written per head into [128, NCH, 65] tiles, col 64 = ones.
                for h in range(HL):
                    nc.sync.dma_start(vt[h][:, :, KC:KC + 2],
                                      _mm(ones8.ap()))
                for c in range(NCH):
                    vt_ps = pa_ps2.tile([128, CL], F32, tag="vt_ps")
                    for k in range(NSUB):
                        nc.tensor.matmul(
                            vt_ps[:],
                            c_sb[:, k, 128 * c:128 * (c + 1)],
                            wv_sb[:, k, :],
                            start=(k == 0), stop=(k == NSUB - 1))
                    for h in range(HL):
                        nc.vector.tensor_tensor(
                            vt[h][:, c, 0:KC], vt_ps[:, KC * h:KC * (h + 1)],
                            bv_sb[:, KC * h:KC * (h + 1)], ALU.add)

            # ---------------- Phase B: attention ----------------
            with tc.tile_pool(name="pb", bufs=1) as pb, \
                 tc.tile_pool(name="pb2", bufs=2) as pb2, \
                 tc.tile_pool(name="pb_ps", bufs=1, space="PSUM") as pb_ps:
                for pair in range(HL // 2):
                    heads = (2 * pair, 2 * pair + 1)
                    es = {h: pb.tile([128, NCH, T], MM_DT, tag=f"expS{h % 2}", name=f"es{h}")
                          for h in heads}
                    av = {h: pb_ps.tile([KC + 2, T], F32, tag=f"av{h % 2}", name=f"av{h}")
                          for h in heads}
                    pd = {h: pb2.tile([NB, T], F32, tag=f"pd{h % 2}", name=f"pd{h}")
                          for h in heads}
                    for h in heads:
                        nc.vector.memset(pd[h][:], 0.0)

                    for c in range(NCH):
                        st = {}
                        for h in heads:
                            rb = 64 * (h % 2)
                            sub = h // 2
                            st[h] = pb_ps.tile([128, T], F32,
                                               tag=f"st{h % 2}", name=f"st{h}")
                            for n in range(2):
                                nc.tensor.matmul(
                                    st[h][:, 512 * n:512 * (n + 1)],
                                    k_sb[rb:rb + 64, sub,
                                         128 * c:128 * (c + 1)],
                                    q_sb[rb:rb + 64, sub,
                                         512 * n:512 * (n + 1)],
                                    start=True, stop=True)
                        t_lo, q0, w = _chunk_window(c)
                        for h in heads:
                            if EN_BANDK:
                                # band_k: staircase window read from rtpad
                                wt = pb2.tile([128, 136], F32,
                                              tag=f"wk{h % 2}", name=f"wt{h}")
                                nc.sync.dma_start(
                                    wt[:, 0:w],
                                    _raw(rtp[h].ap(), t_lo * RT_ROW + 135 - q0,
                                         [[1, 128], [RT_ROW - 1, w]]))
                                nc.vector.tensor_tensor(
                                    st[h][:, t_lo:t_lo + w],
                                    st[h][:, t_lo:t_lo + w],
                                    wt[:, 0:w], ALU.add)
                            # softmax numerator
                            nc.scalar.activation(es[h][:, c, :], st[h][:],
                                                 AF.Exp)
                            # stage expS window, gather 9 diagonals into pd
                            buf = w2[h * NCH + c].ap()
                            if EN_W2:
                                nc.sync.dma_start(
                                    _raw(buf, 8 * W2_COLS + q0,
                                         [[W2_COLS, 128], [1, w]]),
                                    _f32(es[h][:, c, t_lo:t_lo + w]))
                            if EN_GATHER:
                                pdc = pb2.tile([NB, 136], F32,
                                               tag=f"pdc{h % 2}",
                                               name=f"pdc{h}")
                                nc.sync.dma_start(
                                    pdc[:, 0:w],
                                    _raw(buf, q0 * (W2_COLS + 1),
                                         [[W2_COLS, NB], [W2_COLS + 1, w]]))
                                nc.vector.tensor_tensor(
                                    pd[h][:, t_lo:t_lo + w],
                                    pd[h][:, t_lo:t_lo + w],
                                    pdc[:, 0:w], ALU.add)
                            # A @ V (+ ones column -> row 64 = softmax denom)
                            for n in range(2):
                                nc.tensor.matmul(
                                    av[h][:, 512 * n:512 * (n + 1)],
                                    vt[h][:, c, :],
                                    es[h][:, c, 512 * n:512 * (n + 1)],
                                    start=(c == 0), stop=False,
                                    skip_group_check=True)

                    for h in heads:
                        # band_v: out[t] += sum_j pd[j, t] * emb_v[j]
                        for n in range(2):
                            nc.tensor.matmul(
                                av[h][0:KC, 512 * n:512 * (n + 1)],
                                ev_sb[:],
                                pd[h][:, 512 * n:512 * (n + 1)],
                                start=False, stop=True, skip_group_check=True)
                        # normalize by row 64 (denominator) and merge heads
                        rows = 64 * (h % 2)
                        if EN_NORM:
                            rl = pb2.tile([1, T], F32, tag=f"rl{h % 2}",
                                          name=f"rl{h}")
                            nc.vector.reciprocal(rl[:], av[h][KC:KC + 1, :])
                            rlr = pb2.tile([KC, T], F32, tag=f"rlr{h % 2}",
                                           name=f"rlr{h}")
                            nc.gpsimd.partition_broadcast(rlr[:], rl[:])
                            nc.vector.tensor_tensor(
                                merged[rows:rows + KC, h // 2, :],
                                av[h][0:KC, :], rlr[:], ALU.mult)
                        else:
                            nc.vector.tensor_copy(
                                merged[rows:rows + KC, h // 2, :],
                                av[h][0:KC, :])

            # ---------------- Phase C: output projection ----------------
            with tc.tile_pool(name="pc", bufs=3) as pc, \
                 tc.tile_pool(name="pc_ps", bufs=3, space="PSUM") as pc_ps:
                for m in range(NSUB):
                    for n in range(2):
                        ps = pc_ps.tile([128, 512], F32, tag="o_ps")
                        for k in range(LSUB):
                            nc.tensor.matmul(
                                ps[:],
                                wo_sb[:, k, 128 * m:128 * (m + 1)],
                                merged[:, k, 512 * n:512 * (n + 1)],
                                start=(k == 0), stop=(k == LSUB - 1))
                        ot = pc.tile([128, 512], F32, tag="o_sb")
                        nc.vector.tensor_copy(ot[:], ps[:])
                        nc.sync.dma_start(
                            outp.ap()[128 * m:128 * (m + 1),
                                      512 * n:512 * (n + 1)],
                            ot[:])

    nc.compile()
    return nc


_CACHE = {}


def _get_program():
    if "nc" not in _CACHE:
        _CACHE["nc"] = build_program()
    return _CACHE["nc"]


def _prep_core_inputs(core, x, c, Wq, bq, Wk, bk, Wv, bv, Wo,
                      emb_rel_k, emb_rel_v, zeros_rt, zeros_w2):
    b, hg = core // 2, core % 2
    hsl = slice(hg * CL, (hg + 1) * CL)
    scale = KC ** -0.5
    ek = np.ascontiguousarray(emb_rel_k[0])  # [9, 64]
    ekt = np.ascontiguousarray(ek.T)         # [64, 9]
    ins = {
        "ones8": np.concatenate([np.ones((128, NCH, 1), np.float32),
                                 np.zeros((128, NCH, 1), np.float32)], axis=2),
        "xb": np.ascontiguousarray(x[b]),
        "cb": np.ascontiguousarray(c[b]),
        "wqt": np.ascontiguousarray((Wq[hsl] * scale).T),
        "wkt": np.ascontiguousarray(Wk[hsl].T),
        "wvt": np.ascontiguousarray(Wv[hsl].T),
        "wot": np.ascontiguousarray(Wo[:, hsl].T),
        "bq2": np.ascontiguousarray((bq[hsl] * scale).reshape(LSUB, 128).T),
        "bk2": np.ascontiguousarray(bk[hsl].reshape(LSUB, 128).T),
        "bvr": np.ascontiguousarray(np.tile(bv[hsl][None, :], (128, 1))),
        "ekt2": np.ascontiguousarray(np.concatenate([ekt, ekt], axis=0)),
        "ev9": np.ascontiguousarray(emb_rel_v[0]),
    }
    for h in range(HL):
        ins[f"rtp{h}"] = zeros_rt
    for i in range(HL * NCH):
        ins[f"w2_{i}"] = zeros_w2
    return ins


def kernel(**inputs):
    inputs = {k: np.asarray(v, dtype=np.float32) for k, v in inputs.items()}
    nc = _get_program()
    zeros_rt = np.zeros(T * RT_ROW, np.float32)
    zeros_w2 = np.zeros(W2_ROWS * W2_COLS, np.float32)
    in_maps = [
        _prep_core_inputs(
            core, inputs["x"], inputs["c"],
            inputs["Wq"], inputs["bq"], inputs["Wk"], inputs["bk"],
            inputs["Wv"], inputs["bv"], inputs["Wo"],
            inputs["emb_rel_k"], inputs["emb_rel_v"],
            zeros_rt, zeros_w2)
        for core in range(8)
    ]
    res = run_bass_kernel_spmd(nc, in_maps, core_ids=list(range(8)),
                               **_CACHE.get("run_kwargs", {}))
    _CACHE["last_result"] = res
    parts = [r["outp"] for r in res.results]
    bo = inputs["bo"]
    out = np.stack([parts[2 * b] + parts[2 * b + 1] + bo[:, None]
                    for b in range(B)])
    return out.astype(np.float32)



# revision 20
# speedup vs baseline: 1.8678x; 1.8678x over previous
"""Trainium2 Bass kernel for nn_MultiHeadAttention_42752104464925.

Multi-head attention (VITS-style) with windowed relative position embeddings
(window=4, heads_share=True).

Math notes
----------
With L=1024, WIN=4, the relative-key logits term rel_to_abs(q_scaled @ rel_k^T)
is a 9-diagonal band:   scores[t,s] += q_scaled[t] . emb_k[s-t+4]   (|s-t|<=4)
and the relative-value term is:
  out[t] += sum_j p[t, t+j-4] * emb_v[j]   (0 <= t+j-4 < L)

Sharding: 8 cores = 4 batches x 2 head-groups (6 heads each). Each core
computes QKV for its 384 channels, attention for its 6 heads, and a partial
output projection Wo[:, slice] @ merged. Host sums the two partials per batch.

Device layout per head: scores are computed TRANSPOSED (S^T[s,t], s on
partitions) so that A@V needs no transpose of the attention weights; the
softmax denominator L[t] (a partition-dim sum) comes for free from a ones
column appended to the V^T stationary operand (M=66).

Band handling (the skew between [t, j] and [s-chunk, t-window] layouts) goes
through host-zeroed DRAM staging buffers. The descriptor-count-heavy (4-byte
scattered) side of each bounce is placed on the SMALL tensor:
  band-k: RtT [9,T] is skew-WRITTEN (9 batched DMAs, 4B descs) and the big
          [128,136]-per-chunk windows are read back with contiguous 576B runs.
  band-v: expS windows are staged with contiguous 560B-run writes (1/head)
          and only the 9 diagonals are gathered back (4B descs, 1/head).
DMA issue is split across both HWDGE sequencers (SP and Activation) so no
single sequencer serializes the kernel.
"""

import math

import numpy as np

import concourse.bacc as bacc
import concourse.bass as bass
import concourse.mybir as mybir
import concourse.tile as tile
from concourse import library_config
from concourse.bass_utils import run_bass_kernel_spmd

# Problem constants (hardcoded per harness contract).
B, C, T, H, KC, WIN = 4, 768, 1024, 12, 64, 4
HL = 6            # heads per core
CL = HL * KC      # 384 local channels
NSUB = C // 128   # 6 k-subtiles over C
LSUB = CL // 128  # 3 subtiles over CL
NCH = T // 128    # 8 s-chunks
NB = 9            # band width (2*WIN+1)

# band-k staging: WB[h][c*CS + sp*RS + u] = Rt[t0(c)+u, (128c+sp)-(t0+u)+4]
RS = 144
CS = 128 * RS
HS = NCH * CS
# band-v staging: W2[h][c*W2CS + (8+sp)*144 + x] = expS^T[128c+sp, t0(c)+x]
# chunk stride 145*144 makes the diagonal gather's (chunk, x) dims merge
# into one 2D scatter AP (DMA APs allow at most 3 dims incl. partition).
W2CS = 145 * 144
W2HS = NCH * W2CS
ESW = 1056        # padded expS row width; data at [16, 1040)

F32 = mybir.dt.float32
AF = mybir.ActivationFunctionType
ALU = mybir.AluOpType

# Matmul input dtype. float32r streams at full PE rate (1 cyc/row for N>=256)
# with fp32 storage; plain float32 runs at 1/4 rate.
MM_DT = mybir.dt.float32r


def _mm(x):
    return x if MM_DT == F32 else x.bitcast(MM_DT)


def _f32(x):
    return x if MM_DT == F32 else x.bitcast(F32)


def _raw(t_ap, off, dims):
    """Raw element-offset AP into (the tensor behind) an AP."""
    return bass.AP(tensor=t_ap.tensor, offset=t_ap.offset + off, ap=dims)


def _chunk_window(c):
    """Clipped t-window [t_lo, t_lo+w) for s-chunk c; q0 = offset into the
    unclipped 136-wide window starting at t0 = 128c - 4."""
    t0 = 128 * c - WIN
    t_lo = max(0, t0)
    q0 = t_lo - t0
    w = min(T, t0 + 136) - t_lo
    return t_lo, q0, w


def build_program():
    nc = bacc.Bacc("TRN2", target_bir_lowering=False, debug=False,
                   enable_asserts=True)

    # ---- I/O ----
    xb = nc.dram_tensor("xb", [C, T], F32, kind="ExternalInput")
    cb = nc.dram_tensor("cb", [C, T], F32, kind="ExternalInput")
    wqt = nc.dram_tensor("wqt", [C, CL], F32, kind="ExternalInput")
    wkt = nc.dram_tensor("wkt", [C, CL], F32, kind="ExternalInput")
    wvt = nc.dram_tensor("wvt", [C, CL], F32, kind="ExternalInput")
    wot = nc.dram_tensor("wot", [CL, C], F32, kind="ExternalInput")
    bq2 = nc.dram_tensor("bq2", [128, LSUB], F32, kind="ExternalInput")
    bk2 = nc.dram_tensor("bk2", [128, LSUB], F32, kind="ExternalInput")
    bvr = nc.dram_tensor("bvr", [128, CL], F32, kind="ExternalInput")
    ekt2 = nc.dram_tensor("ekt2", [128, NB], F32, kind="ExternalInput")
    ev9 = nc.dram_tensor("ev9", [NB, KC], F32, kind="ExternalInput")
    ones8 = nc.dram_tensor("ones8", [128, NCH, 2], F32, kind="ExternalInput")
    # zero-padded staging buffers (host supplies zeros; device writes data)
    wbk = nc.dram_tensor("wbk", [HL * HS], F32, kind="ExternalInput")
    # one extra chunk of tail padding: the gather's junk columns (x>=136)
    # over-read past the last chunk of the last head
    w2s = nc.dram_tensor("w2s", [HL * W2HS + W2CS], F32, kind="ExternalInput")
    # contiguous j-mirrored copy of RtT (scratch for the band-k skew)
    rtd = nc.dram_tensor("rtd", [HL * NB * ESW], F32, kind="Internal")
    outp = nc.dram_tensor("outp", [C, T], F32, kind="ExternalOutput")

    with tile.TileContext(nc) as tc:
        nc.gpsimd.load_library(library_config.attn)
        with tc.tile_pool(name="persist", bufs=1) as pp:
            # persistent SBUF
            q_sb = pp.tile([128, LSUB, T], MM_DT, tag="q_sb")
            k_sb = pp.tile([128, LSUB, T], MM_DT, tag="k_sb")
            vt = [pp.tile([128, NCH, KC + 2], MM_DT, tag=f"vt{h}", name=f"vt{h}")
                  for h in range(HL)]
            wo_sb = pp.tile([128, LSUB, C], MM_DT, tag="wo_sb")
            merged = pp.tile([128, LSUB, T], MM_DT, tag="merged")
            ekt_sb = pp.tile([128, NB], MM_DT, tag="ekt_sb")
            ev_sb = pp.tile([NB, KC], F32, tag="ev_sb")
            bq_sb = pp.tile([128, LSUB], F32, tag="bq_sb")
            bk_sb = pp.tile([128, LSUB], F32, tag="bk_sb")
            bv_sb = pp.tile([128, CL], F32, tag="bv_sb")

            nc.sync.dma_start(wo_sb[:], _mm(wot.ap().rearrange(
                "(s p) m -> p s m", p=128)))
            nc.sync.dma_start(ekt_sb[:], _mm(ekt2.ap()))
            nc.sync.dma_start(ev_sb[:], ev9.ap())
            nc.sync.dma_start(bq_sb[:], bq2.ap())
            nc.sync.dma_start(bk_sb[:], bk2.ap())
            nc.sync.dma_start(bv_sb[:], bvr.ap())

            # ---------------- Phase A: projections ----------------
            with tc.tile_pool(name="pa", bufs=1) as pa, \
                 tc.tile_pool(name="pa_ps", bufs=3, space="PSUM") as pa_ps, \
                 tc.tile_pool(name="pa_ps2", bufs=2, space="PSUM") as pa_ps2, \
                 tc.tile_pool(name="pa_ps3", bufs=2, space="PSUM") as pa_ps3:
                x_sb = pa.tile([128, NSUB, T], MM_DT, tag="x_sb")
                c_sb = pa.tile([128, NSUB, T], MM_DT, tag="c_sb")
                wq_sb = pa.tile([128, NSUB, CL], MM_DT, tag="wq_sb")
                wk_sb = pa.tile([128, NSUB, CL], MM_DT, tag="wk_sb")
                wv_sb = pa.tile([128, NSUB, CL], MM_DT, tag="wv_sb")
                # RtT[j, h, 16+t] = q_scaled[t] . emb_k[j]; zero margins
                rtsT = pa.tile([NB, HL, ESW], F32, tag="rtsT")
                nc.vector.memset(rtsT[:, :, 0:16], 0.0)
                nc.vector.memset(rtsT[:, :, 16 + T:ESW], 0.0)
                nc.sync.dma_start(x_sb[:], _mm(xb.ap().rearrange(
                    "(s p) t -> p s t", p=128)))
                nc.sync.dma_start(c_sb[:], _mm(cb.ap().rearrange(
                    "(s p) t -> p s t", p=128)))
                nc.sync.dma_start(wq_sb[:], _mm(wqt.ap().rearrange(
                    "(s p) m -> p s m", p=128)))
                nc.sync.dma_start(wk_sb[:], _mm(wkt.ap().rearrange(
                    "(s p) m -> p s m", p=128)))
                nc.sync.dma_start(wv_sb[:], _mm(wvt.ap().rearrange(
                    "(s p) m -> p s m", p=128)))

                # Q and K: out[dl, t] = sum_c W*T[c, dl] * x[c, t]  (+bias)
                for dst, wsb, src, bias in ((q_sb, wq_sb, x_sb, bq_sb),
                                            (k_sb, wk_sb, c_sb, bk_sb)):
                    for m in range(LSUB):
                        for n in range(2):
                            ps = pa_ps.tile([128, 512], F32, tag="qk_ps")
                            for k in range(NSUB):
                                nc.tensor.matmul(
                                    ps[:],
                                    wsb[:, k, 128 * m:128 * (m + 1)],
                                    src[:, k, 512 * n:512 * (n + 1)],
                                    start=(k == 0), stop=(k == NSUB - 1))
                            # fused copy+bias on ACT (idle in phase A)
                            nc.scalar.activation(
                                dst[:, m, 512 * n:512 * (n + 1)], ps[:],
                                AF.Identity, bias=bias[:, m:m + 1])

                # RtT[j, t] per head via 2 matmuls (ekt stationary)
                for h in range(HL):
                    rb = 64 * (h % 2)
                    sub = h // 2
                    for n in range(2):
                        rt_ps = pa_ps3.tile([NB, 512], F32, tag="rt_ps")
                        nc.tensor.matmul(
                            rt_ps[:],
                            ekt_sb[rb:rb + 64, :],
                            q_sb[rb:rb + 64, sub, 512 * n:512 * (n + 1)],
                            start=True, stop=True)
                        nc.scalar.activation(
                            rtsT[:, h, 16 + 512 * n:16 + 512 * (n + 1)],
                            rt_ps[:], AF.Identity)

                # Band-k skew in 3 hops (4B-granular scatter WRITES are ~10x
                # more expensive at the sequencer than scattered reads, so
                # route the skew through reads + a 36B-run write):
                # hop 1: fat j-mirrored copy rtsT -> rtd[(8-j)*ESW + x]
                for j in range(NB):
                    eng = nc.sync if j % 2 == 0 else nc.scalar
                    eng.dma_start(
                        _raw(rtd.ap(), (8 - j) * ESW,
                             [[NB * ESW, HL], [1, ESW]]),
                        rtsT[j:j + 1, :, :])
                # hop 2: anti-diagonal run read RUN[tau, c, k] =
                #        rtd[k*(ESW+1) + 128c + tau + 12]
                run = [pa.tile([128, NCH, 12], F32, tag=f"run{h % 2}",
                               name=f"run{h}") for h in range(HL)]
                for h in range(HL):
                    for k in range(NB):
                        eng = nc.sync if (h * NB + k) % 2 == 0 else nc.scalar
                        eng.dma_start(
                            run[h][:, :, k:k + 1],
                            _raw(rtd.ap(), h * NB * ESW + 12 + (ESW + 1) * k,
                                 [[1, 128], [128, NCH]]))
                # hop 3: 36B-run scatter write into the window staging:
                # WB[h*HS + c*CS + 145*tau + k] = RUN[tau, c, k]
                for h in range(HL):
                    eng = nc.sync if h % 2 == 0 else nc.scalar
                    eng.dma_start(
                        _raw(wbk.ap(), h * HS,
                             [[RS + 1, 128], [CS, NCH], [1, NB]]),
                        run[h][:, :, 0:NB])

                # V^T: out[s, dl] = sum_c c_b[c, s] * WvT[c, dl] (+bias),
                # written per head into [128, NCH, 66] tiles, col 64 = ones.
                for h in range(HL):
                    nc.sync.dma_start(vt[h][:, :, KC:KC + 2],
                                      _mm(ones8.ap()))
                for c in range(NCH):
                    vt_ps = pa_ps2.tile([128, CL], F32, tag="vt_ps")
                    for k in range(NSUB):
                        nc.tensor.matmul(
                            vt_ps[:],
                            c_sb[:, k, 128 * c:128 * (c + 1)],
                            wv_sb[:, k, :],
                            start=(k == 0), stop=(k == NSUB - 1))
                    for h in range(HL):
                        nc.vector.tensor_tensor(
                            vt[h][:, c, 0:KC], vt_ps[:, KC * h:KC * (h + 1)],
                            bv_sb[:, KC * h:KC * (h + 1)], ALU.add)

            # ---------------- Phase B: attention ----------------
            with tc.tile_pool(name="pb", bufs=1) as pb, \
                 tc.tile_pool(name="pb2", bufs=1) as pb2, \
                 tc.tile_pool(name="pb_ps", bufs=1, space="PSUM") as pb_ps:
                # contiguous read-back of the band-k windows, one per head
                wband = [pb.tile([128, NCH, RS], F32, tag=f"wband{h}",
                                 name=f"wband{h}") for h in range(HL)]
                for h in range(HL):
                    nc.sync.dma_start(
                        wband[h][:],
                        _raw(wbk.ap(), h * HS,
                             [[RS, 128], [CS, NCH], [1, RS]]))

                for pair in range(HL // 2):
                    heads = (2 * pair, 2 * pair + 1)
                    es = {h: pb.tile([128, NCH, ESW], MM_DT,
                                     tag=f"expS{h % 2}", name=f"es{h}")
                          for h in heads}
                    av = {h: pb_ps.tile([KC + 2, T], F32, tag=f"av{h % 2}",
                                        name=f"av{h}")
                          for h in heads}
                    pdr = {h: pb2.tile([NB, NCH, RS], F32, tag=f"pdr{h % 2}",
                                       name=f"pdr{h}")
                           for h in heads}
                    pd = {h: pb2.tile([NB, T], F32, tag=f"pd{h % 2}",
                                      name=f"pd{h}")
                          for h in heads}
                    for h in heads:
                        nc.vector.memset(pd[h][:], 0.0)
                        nc.vector.memset(_f32(es[h][:, :, 0:16]), 0.0)
                        nc.vector.memset(_f32(es[h][:, :, 16 + T:ESW]), 0.0)

                    for c in range(NCH):
                        t_lo, q0, w = _chunk_window(c)
                        for h in heads:
                            rb = 64 * (h % 2)
                            sub = h // 2
                            for n in range(2):
                                # 512-wide st halves: 4 concurrent
                                # MM->add->exp->MM chains per chunk
                                stn = pb_ps.tile(
                                    [128, 512], F32,
                                    tag=f"st{h % 2}n{n}", name=f"st{h}n{n}")
                                nc.tensor.matmul(
                                    stn[:],
                                    k_sb[rb:rb + 64, sub,
                                         128 * c:128 * (c + 1)],
                                    q_sb[rb:rb + 64, sub,
                                         512 * n:512 * (n + 1)],
                                    start=True, stop=True)
                                # band_k add from the staged window
                                a = max(t_lo, 512 * n)
                                b = min(t_lo + w, 512 * (n + 1))
                                if a < b:
                                    nc.vector.tensor_tensor(
                                        stn[:, a - 512 * n:b - 512 * n],
                                        stn[:, a - 512 * n:b - 512 * n],
                                        wband[h][:, c,
                                                 q0 + a - t_lo:q0 + b - t_lo],
                                        ALU.add)
                                # softmax numerator
                                nc.scalar.activation(
                                    es[h][:, c,
                                          16 + 512 * n:16 + 512 * (n + 1)],
                                    stn[:], AF.Exp)
                                # A @ V (+ ones col -> row 64 = denominator)
                                nc.tensor.matmul(
                                    av[h][:, 512 * n:512 * (n + 1)],
                                    vt[h][:, c, :],
                                    es[h][:, c,
                                          16 + 512 * n:16 + 512 * (n + 1)],
                                    start=(c == 0), stop=False,
                                    skip_group_check=True)

                    for h in heads:
                        # stage all 8 expS windows with contiguous runs
                        es_full = es[h][:]
                        src = bass.AP(
                            tensor=es_full.tensor, offset=es_full.offset + 12,
                            ap=[list(es_full.ap[0]),
                                [ESW + 128, NCH], [1, 140]])
                        nc.sync.dma_start(
                            _raw(w2s.ap(), h * W2HS + 8 * 144,
                                 [[144, 128], [W2CS, NCH], [1, 140]]),
                            _f32(src))
                        # gather the 9 diagonals of each chunk window; the
                        # junk tail (x in [136,144)) lands in an unread
                        # region of pdr
                        eng = nc.sync if h % 2 == 0 else nc.scalar
                        eng.dma_start(
                            pdr[h][:],
                            _raw(w2s.ap(), h * W2HS,
                                 [[144, NB], [145, NCH * 144]]))
                        for c in range(NCH):
                            t_lo, q0, w = _chunk_window(c)
                            nc.vector.tensor_tensor(
                                pd[h][:, t_lo:t_lo + w],
                                pd[h][:, t_lo:t_lo + w],
                                pdr[h][:, c, q0:q0 + w], ALU.add)
                        # band_v: out[t] += sum_j pd[j, t] * emb_v[j]
                        for n in range(2):
                            nc.tensor.matmul(
                                av[h][0:KC, 512 * n:512 * (n + 1)],
                                ev_sb[:],
                                pd[h][:, 512 * n:512 * (n + 1)],
                                start=False, stop=True, skip_group_check=True)
                        # normalize by row 64 (denominator) and merge heads
                        rows = 64 * (h % 2)
                        rl = pb2.tile([1, T], F32, tag=f"rl{h % 2}",
                                      name=f"rl{h}")
                        nc.vector.reciprocal(rl[:], av[h][KC:KC + 1, :])
                        rlr = pb2.tile([KC, T], F32, tag=f"rlr{h % 2}",
                                       name=f"rlr{h}")
                        nc.gpsimd.partition_broadcast(rlr[:], rl[:])
                        nc.vector.tensor_tensor(
                            merged[rows:rows + KC, h // 2, :],
                            av[h][0:KC, :], rlr[:], ALU.mult)

            # ---------------- Phase C: output projection ----------------
            with tc.tile_pool(name="pc", bufs=3) as pc, \
                 tc.tile_pool(name="pc_ps", bufs=3, space="PSUM") as pc_ps:
                for m in range(NSUB):
                    for n in range(2):
                        ps = pc_ps.tile([128, 512], F32, tag="o_ps")
                        for k in range(LSUB):
                            nc.tensor.matmul(
                                ps[:],
                                wo_sb[:, k, 128 * m:128 * (m + 1)],
                                merged[:, k, 512 * n:512 * (n + 1)],
                                start=(k == 0), stop=(k == LSUB - 1))
                        ot = pc.tile([128, 512], F32, tag="o_sb")
                        nc.vector.tensor_copy(ot[:], ps[:])
                        nc.sync.dma_start(
                            outp.ap()[128 * m:128 * (m + 1),
                                      512 * n:512 * (n + 1)],
                            ot[:])

    nc.compile()
    return nc


_CACHE = {}


def _get_program():
    if "nc" not in _CACHE:
        _CACHE["nc"] = build_program()
    return _CACHE["nc"]


def _prep_core_inputs(core, x, c, Wq, bq, Wk, bk, Wv, bv, Wo,
                      emb_rel_k, emb_rel_v, zeros_wbk, zeros_w2s):
    b, hg = core // 2, core % 2
    hsl = slice(hg * CL, (hg + 1) * CL)
    scale = KC ** -0.5
    ek = np.ascontiguousarray(emb_rel_k[0])  # [9, 64]
    ekt = np.ascontiguousarray(ek.T)         # [64, 9]
    return {
        "xb": np.ascontiguousarray(x[b]),
        "cb": np.ascontiguousarray(c[b]),
        "wqt": np.ascontiguousarray((Wq[hsl] * scale).T),
        "wkt": np.ascontiguousarray(Wk[hsl].T),
        "wvt": np.ascontiguousarray(Wv[hsl].T),
        "wot": np.ascontiguousarray(Wo[:, hsl].T),
        "bq2": np.ascontiguousarray((bq[hsl] * scale).reshape(LSUB, 128).T),
        "bk2": np.ascontiguousarray(bk[hsl].reshape(LSUB, 128).T),
        "bvr": np.ascontiguousarray(np.tile(bv[hsl][None, :], (128, 1))),
        "ekt2": np.ascontiguousarray(np.concatenate([ekt, ekt], axis=0)),
        "ev9": np.ascontiguousarray(emb_rel_v[0]),
        "ones8": np.concatenate([np.ones((128, NCH, 1), np.float32),
                                 np.zeros((128, NCH, 1), np.float32)], axis=2),
        "wbk": zeros_wbk,
        "w2s": zeros_w2s,
    }


def kernel(**inputs):
    inputs = {k: np.asarray(v, dtype=np.float32) for k, v in inputs.items()}
    nc = _get_program()
    zeros_wbk = np.zeros(HL * HS, np.float32)
    zeros_w2s = np.zeros(HL * W2HS + W2CS, np.float32)
    in_maps = [
        _prep_core_inputs(
            core, inputs["x"], inputs["c"],
            inputs["Wq"], inputs["bq"], inputs["Wk"], inputs["bk"],
            inputs["Wv"], inputs["bv"], inputs["Wo"],
            inputs["emb_rel_k"], inputs["emb_rel_v"],
            zeros_wbk, zeros_w2s)
        for core in range(8)
    ]
    res = run_bass_kernel_spmd(nc, in_maps, core_ids=list(range(8)),
                               **_CACHE.get("run_kwargs", {}))
    _CACHE["last_result"] = res
    parts = [r["outp"] for r in res.results]
    bo = inputs["bo"]
    out = np.stack([parts[2 * b] + parts[2 * b + 1] + bo[:, None]
                    for b in range(B)])
    return out.astype(np.float32)


# revision 22
# speedup vs baseline: 1.9705x; 1.0550x over previous
"""Trainium2 Bass kernel for nn_MultiHeadAttention_42752104464925.

Multi-head attention (VITS-style) with windowed relative position embeddings
(window=4, heads_share=True).

Math notes
----------
With L=1024, WIN=4, the relative-key logits term rel_to_abs(q_scaled @ rel_k^T)
is a 9-diagonal band:   scores[t,s] += q_scaled[t] . emb_k[s-t+4]   (|s-t|<=4)
and the relative-value term is:
  out[t] += sum_j p[t, t+j-4] * emb_v[j]   (0 <= t+j-4 < L)

Sharding: 8 cores = 4 batches x 2 head-groups (6 heads each). Each core
computes QKV for its 384 channels, attention for its 6 heads, and a partial
output projection Wo[:, slice] @ merged. Host sums the two partials per batch.

Device layout per head: scores are computed TRANSPOSED (S^T[s,t], s on
partitions) so that A@V needs no transpose of the attention weights; the
softmax denominator L[t] (a partition-dim sum) comes for free from a ones
column appended to the V^T stationary operand (M=66).

Band handling (the skew between [t, j] and [s-chunk, t-window] layouts) goes
through host-zeroed DRAM staging buffers. The descriptor-count-heavy (4-byte
scattered) side of each bounce is placed on the SMALL tensor:
  band-k: RtT [9,T] is skew-WRITTEN (9 batched DMAs, 4B descs) and the big
          [128,136]-per-chunk windows are read back with contiguous 576B runs.
  band-v: expS windows are staged with contiguous 560B-run writes (1/head)
          and only the 9 diagonals are gathered back (4B descs, 1/head).
DMA issue is split across both HWDGE sequencers (SP and Activation) so no
single sequencer serializes the kernel.
"""

import math

import numpy as np

import concourse.bacc as bacc
import concourse.bass as bass
import concourse.mybir as mybir
import concourse.tile as tile
from concourse import library_config
from concourse.bass_utils import run_bass_kernel_spmd

# Problem constants (hardcoded per harness contract).
B, C, T, H, KC, WIN = 4, 768, 1024, 12, 64, 4
HL = 6            # heads per core
CL = HL * KC      # 384 local channels
NSUB = C // 128   # 6 k-subtiles over C
LSUB = CL // 128  # 3 subtiles over CL
NCH = T // 128    # 8 s-chunks
NB = 9            # band width (2*WIN+1)

# band-k staging: WB[h][c*CS + sp*RS + u] = Rt[t0(c)+u, (128c+sp)-(t0+u)+4]
RS = 144
CS = 128 * RS
HS = NCH * CS
# band-v staging: W2[h][c*W2CS + (8+sp)*144 + x] = expS^T[128c+sp, t0(c)+x]
# chunk stride 145*144 makes the diagonal gather's (chunk, x) dims merge
# into one 2D scatter AP (DMA APs allow at most 3 dims incl. partition).
W2CS = 145 * 144
W2HS = NCH * W2CS
ESW = 1056        # padded expS row width; data at [16, 1040)

F32 = mybir.dt.float32
AF = mybir.ActivationFunctionType
ALU = mybir.AluOpType

# Matmul input dtype. float32r streams at full PE rate (1 cyc/row for N>=256)
# with fp32 storage; plain float32 runs at 1/4 rate.
MM_DT = mybir.dt.float32r


def _mm(x):
    return x if MM_DT == F32 else x.bitcast(MM_DT)


def _f32(x):
    return x if MM_DT == F32 else x.bitcast(F32)


def _raw(t_ap, off, dims):
    """Raw element-offset AP into (the tensor behind) an AP."""
    return bass.AP(tensor=t_ap.tensor, offset=t_ap.offset + off, ap=dims)


def _chunk_window(c):
    """Clipped t-window [t_lo, t_lo+w) for s-chunk c; q0 = offset into the
    unclipped 136-wide window starting at t0 = 128c - 4."""
    t0 = 128 * c - WIN
    t_lo = max(0, t0)
    q0 = t_lo - t0
    w = min(T, t0 + 136) - t_lo
    return t_lo, q0, w


def build_program():
    nc = bacc.Bacc("TRN2", target_bir_lowering=False, debug=False,
                   enable_asserts=True)

    # ---- I/O ----
    xb = nc.dram_tensor("xb", [C, T], F32, kind="ExternalInput")
    cb = nc.dram_tensor("cb", [C, T], F32, kind="ExternalInput")
    wqt = nc.dram_tensor("wqt", [C, CL], F32, kind="ExternalInput")
    wkt = nc.dram_tensor("wkt", [C, CL], F32, kind="ExternalInput")
    wvt = nc.dram_tensor("wvt", [C, CL], F32, kind="ExternalInput")
    wot = nc.dram_tensor("wot", [CL, C], F32, kind="ExternalInput")
    bq2 = nc.dram_tensor("bq2", [128, LSUB], F32, kind="ExternalInput")
    bk2 = nc.dram_tensor("bk2", [128, LSUB], F32, kind="ExternalInput")
    bvr = nc.dram_tensor("bvr", [128, CL], F32, kind="ExternalInput")
    ekt2 = nc.dram_tensor("ekt2", [128, NB], F32, kind="ExternalInput")
    ev9 = nc.dram_tensor("ev9", [NB, KC], F32, kind="ExternalInput")
    ones8 = nc.dram_tensor("ones8", [128, NCH, 2], F32, kind="ExternalInput")
    # zero-padded staging buffers (host supplies zeros; device writes data)
    wbk = nc.dram_tensor("wbk", [HL * HS], F32, kind="ExternalInput")
    # one extra chunk of tail padding: the gather's junk columns (x>=136)
    # over-read past the last chunk of the last head
    w2s = nc.dram_tensor("w2s", [HL * W2HS + W2CS], F32, kind="ExternalInput")
    # contiguous j-mirrored copy of RtT (scratch for the band-k skew)
    rtd = nc.dram_tensor("rtd", [HL * NB * ESW], F32, kind="Internal")
    outp = nc.dram_tensor("outp", [C, T], F32, kind="ExternalOutput")

    with tile.TileContext(nc) as tc:
        nc.gpsimd.load_library(library_config.attn)
        with tc.tile_pool(name="persist", bufs=1) as pp:
            # persistent SBUF
            q_sb = pp.tile([128, LSUB, T], MM_DT, tag="q_sb")
            k_sb = pp.tile([128, LSUB, T], MM_DT, tag="k_sb")
            vt = [pp.tile([128, NCH, KC + 2], MM_DT, tag=f"vt{h}", name=f"vt{h}")
                  for h in range(HL)]
            wo_sb = pp.tile([128, LSUB, C], MM_DT, tag="wo_sb")
            merged = pp.tile([128, LSUB, T], MM_DT, tag="merged")
            ekt_sb = pp.tile([128, NB], MM_DT, tag="ekt_sb")
            ev_sb = pp.tile([NB, KC], F32, tag="ev_sb")
            bq_sb = pp.tile([128, LSUB], F32, tag="bq_sb")
            bk_sb = pp.tile([128, LSUB], F32, tag="bk_sb")
            bv_sb = pp.tile([128, CL], F32, tag="bv_sb")

            nc.sync.dma_start(wo_sb[:], _mm(wot.ap().rearrange(
                "(s p) m -> p s m", p=128)))
            nc.sync.dma_start(ekt_sb[:], _mm(ekt2.ap()))
            nc.sync.dma_start(ev_sb[:], ev9.ap())
            nc.sync.dma_start(bq_sb[:], bq2.ap())
            nc.sync.dma_start(bk_sb[:], bk2.ap())
            nc.sync.dma_start(bv_sb[:], bvr.ap())

            # ---------------- Phase A: projections ----------------
            with tc.tile_pool(name="pa", bufs=1) as pa, \
                 tc.tile_pool(name="pa_ps", bufs=3, space="PSUM") as pa_ps, \
                 tc.tile_pool(name="pa_ps2", bufs=2, space="PSUM") as pa_ps2, \
                 tc.tile_pool(name="pa_ps3", bufs=2, space="PSUM") as pa_ps3:
                x_sb = pa.tile([128, NSUB, T], MM_DT, tag="x_sb")
                c_sb = pa.tile([128, NSUB, T], MM_DT, tag="c_sb")
                wq_sb = pa.tile([128, NSUB, CL], MM_DT, tag="wq_sb")
                wk_sb = pa.tile([128, NSUB, CL], MM_DT, tag="wk_sb")
                wv_sb = pa.tile([128, NSUB, CL], MM_DT, tag="wv_sb")
                # RtT[j, h, 16+t] = q_scaled[t] . emb_k[j]; zero margins
                rtsT = pa.tile([NB, HL, ESW], F32, tag="rtsT")
                nc.vector.memset(rtsT[:, :, 0:16], 0.0)
                nc.vector.memset(rtsT[:, :, 16 + T:ESW], 0.0)
                nc.sync.dma_start(x_sb[:], _mm(xb.ap().rearrange(
                    "(s p) t -> p s t", p=128)))
                nc.sync.dma_start(c_sb[:], _mm(cb.ap().rearrange(
                    "(s p) t -> p s t", p=128)))
                nc.sync.dma_start(wq_sb[:], _mm(wqt.ap().rearrange(
                    "(s p) m -> p s m", p=128)))
                nc.sync.dma_start(wk_sb[:], _mm(wkt.ap().rearrange(
                    "(s p) m -> p s m", p=128)))
                nc.sync.dma_start(wv_sb[:], _mm(wvt.ap().rearrange(
                    "(s p) m -> p s m", p=128)))

                # Q and K: out[dl, t] = sum_c W*T[c, dl] * x[c, t]  (+bias)
                for dst, wsb, src, bias in ((q_sb, wq_sb, x_sb, bq_sb),
                                            (k_sb, wk_sb, c_sb, bk_sb)):
                    for m in range(LSUB):
                        for n in range(2):
                            ps = pa_ps.tile([128, 512], F32, tag="qk_ps")
                            for k in range(NSUB):
                                nc.tensor.matmul(
                                    ps[:],
                                    wsb[:, k, 128 * m:128 * (m + 1)],
                                    src[:, k, 512 * n:512 * (n + 1)],
                                    start=(k == 0), stop=(k == NSUB - 1))
                            # fused copy+bias on ACT (idle in phase A)
                            nc.scalar.activation(
                                dst[:, m, 512 * n:512 * (n + 1)], ps[:],
                                AF.Identity, bias=bias[:, m:m + 1])

                # RtT[j, t] per head via 2 matmuls (ekt stationary)
                for h in range(HL):
                    rb = 64 * (h % 2)
                    sub = h // 2
                    for n in range(2):
                        rt_ps = pa_ps3.tile([NB, 512], F32, tag="rt_ps")
                        nc.tensor.matmul(
                            rt_ps[:],
                            ekt_sb[rb:rb + 64, :],
                            q_sb[rb:rb + 64, sub, 512 * n:512 * (n + 1)],
                            start=True, stop=True)
                        nc.scalar.activation(
                            rtsT[:, h, 16 + 512 * n:16 + 512 * (n + 1)],
                            rt_ps[:], AF.Identity)

                # Band-k skew in 3 hops (4B-granular scatter WRITES are ~10x
                # more expensive at the sequencer than scattered reads, so
                # route the skew through reads + a 36B-run write):
                # hop 1: fat j-mirrored copy rtsT -> rtd[(8-j)*ESW + x]
                for j in range(NB):
                    eng = nc.sync if j % 2 == 0 else nc.scalar
                    eng.dma_start(
                        _raw(rtd.ap(), (8 - j) * ESW,
                             [[NB * ESW, HL], [1, ESW]]),
                        rtsT[j:j + 1, :, :])
                # hop 2: anti-diagonal run read RUN[tau, c, k] =
                #        rtd[k*(ESW+1) + 128c + tau + 12]
                run = [pa.tile([128, NCH, 12], F32, tag=f"run{h % 2}",
                               name=f"run{h}") for h in range(HL)]
                for h in range(HL):
                    for k in range(NB):
                        eng = nc.sync if (h * NB + k) % 2 == 0 else nc.scalar
                        eng.dma_start(
                            run[h][:, :, k:k + 1],
                            _raw(rtd.ap(), h * NB * ESW + 12 + (ESW + 1) * k,
                                 [[1, 128], [128, NCH]]))
                # hop 3: 36B-run scatter write into the window staging:
                # WB[h*HS + c*CS + 145*tau + k] = RUN[tau, c, k]
                for h in range(HL):
                    eng = nc.sync if h % 2 == 0 else nc.scalar
                    eng.dma_start(
                        _raw(wbk.ap(), h * HS,
                             [[RS + 1, 128], [CS, NCH], [1, NB]]),
                        run[h][:, :, 0:NB])

                # V^T: out[s, dl] = sum_c c_b[c, s] * WvT[c, dl] (+bias),
                # written per head into [128, NCH, 66] tiles, col 64 = ones.
                for h in range(HL):
                    nc.scalar.dma_start(vt[h][:, :, KC:KC + 2],
                                        _mm(ones8.ap()))
                for c in range(NCH):
                    vt_ps = pa_ps2.tile([128, CL], F32, tag="vt_ps")
                    for k in range(NSUB):
                        nc.tensor.matmul(
                            vt_ps[:],
                            c_sb[:, k, 128 * c:128 * (c + 1)],
                            wv_sb[:, k, :],
                            start=(k == 0), stop=(k == NSUB - 1))
                    for h in range(HL):
                        nc.vector.tensor_tensor(
                            vt[h][:, c, 0:KC], vt_ps[:, KC * h:KC * (h + 1)],
                            bv_sb[:, KC * h:KC * (h + 1)], ALU.add)

            # ---------------- Phase B: attention ----------------
            with tc.tile_pool(name="pb", bufs=1) as pb, \
                 tc.tile_pool(name="pb2", bufs=1) as pb2, \
                 tc.tile_pool(name="pb_ps", bufs=1, space="PSUM") as pb_ps:
                # contiguous read-back of the band-k windows, one per head
                wband = [pb.tile([128, NCH, RS], F32, tag=f"wband{h}",
                                 name=f"wband{h}") for h in range(HL)]
                for h in range(HL):
                    nc.sync.dma_start(
                        wband[h][:],
                        _raw(wbk.ap(), h * HS,
                             [[RS, 128], [CS, NCH], [1, RS]]))

                for pair in range(HL // 2):
                    heads = (2 * pair, 2 * pair + 1)
                    es = {h: pb.tile([128, NCH, ESW], MM_DT,
                                     tag=f"expS{h % 2}", name=f"es{h}")
                          for h in heads}
                    av = {h: pb_ps.tile([KC + 2, T], F32, tag=f"av{h % 2}",
                                        name=f"av{h}")
                          for h in heads}
                    pdr = {h: pb2.tile([NB, NCH, RS], F32, tag=f"pdr{h % 2}",
                                       name=f"pdr{h}")
                           for h in heads}
                    pd = {h: pb2.tile([NB, T], F32, tag=f"pd{h % 2}",
                                      name=f"pd{h}")
                          for h in heads}
                    for h in heads:
                        nc.vector.memset(pd[h][:], 0.0)
                        nc.vector.memset(_f32(es[h][:, :, 0:16]), 0.0)
                        nc.vector.memset(_f32(es[h][:, :, 16 + T:ESW]), 0.0)

                    for c in range(NCH):
                        t_lo, q0, w = _chunk_window(c)
                        for h in heads:
                            rb = 64 * (h % 2)
                            sub = h // 2
                            for n in range(2):
                                # 512-wide st halves: 4 concurrent
                                # MM->add->exp->MM chains per chunk
                                stn = pb_ps.tile(
                                    [128, 512], F32,
                                    tag=f"st{h % 2}n{n}", name=f"st{h}n{n}")
                                nc.tensor.matmul(
                                    stn[:],
                                    k_sb[rb:rb + 64, sub,
                                         128 * c:128 * (c + 1)],
                                    q_sb[rb:rb + 64, sub,
                                         512 * n:512 * (n + 1)],
                                    start=True, stop=True)
                                # band_k add from the staged window
                                a = max(t_lo, 512 * n)
                                b = min(t_lo + w, 512 * (n + 1))
                                if a < b:
                                    nc.vector.tensor_tensor(
                                        stn[:, a - 512 * n:b - 512 * n],
                                        stn[:, a - 512 * n:b - 512 * n],
                                        wband[h][:, c,
                                                 q0 + a - t_lo:q0 + b - t_lo],
                                        ALU.add)
                                # softmax numerator
                                nc.scalar.activation(
                                    es[h][:, c,
                                          16 + 512 * n:16 + 512 * (n + 1)],
                                    stn[:], AF.Exp)
                                # A @ V (+ ones col -> row 64 = denominator)
                                nc.tensor.matmul(
                                    av[h][:, 512 * n:512 * (n + 1)],
                                    vt[h][:, c, :],
                                    es[h][:, c,
                                          16 + 512 * n:16 + 512 * (n + 1)],
                                    start=(c == 0), stop=False,
                                    skip_group_check=True)

                    for h in heads:
                        # stage all 8 expS windows with contiguous runs
                        es_full = es[h][:]
                        src = bass.AP(
                            tensor=es_full.tensor, offset=es_full.offset + 12,
                            ap=[list(es_full.ap[0]),
                                [ESW + 128, NCH], [1, 140]])
                        nc.sync.dma_start(
                            _raw(w2s.ap(), h * W2HS + 8 * 144,
                                 [[144, 128], [W2CS, NCH], [1, 140]]),
                            _f32(src))
                        # gather the 9 diagonals of each chunk window; the
                        # junk tail (x in [136,144)) lands in an unread
                        # region of pdr
                        eng = nc.sync if h % 2 == 0 else nc.scalar
                        eng.dma_start(
                            pdr[h][:],
                            _raw(w2s.ap(), h * W2HS,
                                 [[144, NB], [145, NCH * 144]]))
                        for c in range(NCH):
                            t_lo, q0, w = _chunk_window(c)
                            nc.vector.tensor_tensor(
                                pd[h][:, t_lo:t_lo + w],
                                pd[h][:, t_lo:t_lo + w],
                                pdr[h][:, c, q0:q0 + w], ALU.add)
                        # band_v: out[t] += sum_j pd[j, t] * emb_v[j]
                        for n in range(2):
                            nc.tensor.matmul(
                                av[h][0:KC, 512 * n:512 * (n + 1)],
                                ev_sb[:],
                                pd[h][:, 512 * n:512 * (n + 1)],
                                start=False, stop=True, skip_group_check=True)
                        # normalize by row 64 (denominator) and merge heads
                        rows = 64 * (h % 2)
                        # copy the denominator row to SBUF, then the fast
                        # approx reciprocal (~5x cheaper than reciprocal())
                        rl0 = pb2.tile([1, T], F32, tag=f"rl0{h % 2}",
                                       name=f"rl0{h}")
                        nc.vector.tensor_copy(rl0[:], av[h][KC:KC + 1, :])
                        rl = pb2.tile([1, T], F32, tag=f"rl{h % 2}",
                                      name=f"rl{h}")
                        nc.vector.reciprocal_approx_fast(rl[:], rl0[:])
                        rlr = pb2.tile([KC, T], F32, tag=f"rlr{h % 2}",
                                       name=f"rlr{h}")
                        nc.gpsimd.partition_broadcast(rlr[:], rl[:])
                        nc.vector.tensor_tensor(
                            merged[rows:rows + KC, h // 2, :],
                            av[h][0:KC, :], rlr[:], ALU.mult)

            # ---------------- Phase C: output projection ----------------
            with tc.tile_pool(name="pc", bufs=3) as pc, \
                 tc.tile_pool(name="pc_ps", bufs=3, space="PSUM") as pc_ps:
                for m in range(NSUB):
                    for n in range(2):
                        ps = pc_ps.tile([128, 512], F32, tag="o_ps")
                        for k in range(LSUB):
                            nc.tensor.matmul(
                                ps[:],
                                wo_sb[:, k, 128 * m:128 * (m + 1)],
                                merged[:, k, 512 * n:512 * (n + 1)],
                                start=(k == 0), stop=(k == LSUB - 1))
                        ot = pc.tile([128, 512], F32, tag="o_sb")
                        nc.vector.tensor_copy(ot[:], ps[:])
                        nc.sync.dma_start(
                            outp.ap()[128 * m:128 * (m + 1),
                                      512 * n:512 * (n + 1)],
                            ot[:])

    nc.compile()
    return nc


_CACHE = {}


def _get_program():
    if "nc" not in _CACHE:
        _CACHE["nc"] = build_program()
    return _CACHE["nc"]


def _prep_core_inputs(core, x, c, Wq, bq, Wk, bk, Wv, bv, Wo,
                      emb_rel_k, emb_rel_v, zeros_wbk, zeros_w2s):
    b, hg = core // 2, core % 2
    hsl = slice(hg * CL, (hg + 1) * CL)
    scale = KC ** -0.5
    ek = np.ascontiguousarray(emb_rel_k[0])  # [9, 64]
    ekt = np.ascontiguousarray(ek.T)         # [64, 9]
    return {
        "xb": np.ascontiguousarray(x[b]),
        "cb": np.ascontiguousarray(c[b]),
        "wqt": np.ascontiguousarray((Wq[hsl] * scale).T),
        "wkt": np.ascontiguousarray(Wk[hsl].T),
        "wvt": np.ascontiguousarray(Wv[hsl].T),
        "wot": np.ascontiguousarray(Wo[:, hsl].T),
        "bq2": np.ascontiguousarray((bq[hsl] * scale).reshape(LSUB, 128).T),
        "bk2": np.ascontiguousarray(bk[hsl].reshape(LSUB, 128).T),
        "bvr": np.ascontiguousarray(np.tile(bv[hsl][None, :], (128, 1))),
        "ekt2": np.ascontiguousarray(np.concatenate([ekt, ekt], axis=0)),
        "ev9": np.ascontiguousarray(emb_rel_v[0]),
        "ones8": np.concatenate([np.ones((128, NCH, 1), np.float32),
                                 np.zeros((128, NCH, 1), np.float32)], axis=2),
        "wbk": zeros_wbk,
        "w2s": zeros_w2s,
    }


def kernel(**inputs):
    inputs = {k: np.asarray(v, dtype=np.float32) for k, v in inputs.items()}
    nc = _get_program()
    zeros_wbk = np.zeros(HL * HS, np.float32)
    zeros_w2s = np.zeros(HL * W2HS + W2CS, np.float32)
    in_maps = [
        _prep_core_inputs(
            core, inputs["x"], inputs["c"],
            inputs["Wq"], inputs["bq"], inputs["Wk"], inputs["bk"],
            inputs["Wv"], inputs["bv"], inputs["Wo"],
            inputs["emb_rel_k"], inputs["emb_rel_v"],
            zeros_wbk, zeros_w2s)
        for core in range(8)
    ]
    res = run_bass_kernel_spmd(nc, in_maps, core_ids=list(range(8)),
                               **_CACHE.get("run_kwargs", {}))
    _CACHE["last_result"] = res
    parts = [r["outp"] for r in res.results]
    bo = inputs["bo"]
    out = np.stack([parts[2 * b] + parts[2 * b + 1] + bo[:, None]
                    for b in range(B)])
    return out.astype(np.float32)
